# revision 31
# baseline (speedup 1.0000x reference)
# Trainium2 Bass kernel for nn_MEMORY_34986803593776 (scatter_memory).
#
# Math (per sample b):
#   w        = softmax(ck @ mk^T)                             [M]
#   c0       = qa * sigmoid(mem0 @ Wc0 + bc0)                 [DQA]
#   gate     = sigmoid(c0 @ Wm1 + bm1)                        [M*DV]
#   memPre   = mem0 * gate                                    [M*DV]
#   erase    = sig(sig(c0@We+be) + sig(memPre@Wemv+bemv))     [DV]
#   zt       = sig((c0@Wz+bz) + (memPre@Wzmv+bzmv))           [DV]
#   add      = tanh(tanh(zt@Wza+bza) + tanh(memPre@Wamv+bamv))[DV]
#   new      = memPre*(1 - w[m]*erase[dv]) + w[m]*add[dv]     [M,DV]
#
# Sharding: pure data parallel over batch B=16384 across 8 cores (2048/core).
#
# v2 layout strategy:
#  - Host pre-transposes mem to feature-major [f, b] bf16 tiles, so every
#    f-contraction GEMM (c0, gate, emv/zmv, amv) consumes it directly with
#    zero on-device input transposes.  qa/ck also arrive pre-transposed.
#  - Only memPre is PE-transposed back to batch-major for the combine;
#    PSUM->SBUF drains are spread across Act/DVE.
#  - The big elementwise passes (memPre mult + 4 combine passes) are
#    column-split between DVE and Pool(gpsimd).
#  - All DMAs are same-dtype bf16 on the SP queue (HWDGE), keeping the
#    Pool queue free for compute; output is bf16, upcast on host.

import numpy as np
import ml_dtypes

B = 16384
M = 64
DV = 64
DK = 64
DQA = 128
F = M * DV  # 4096
N_CORES = 8
B_CORE = B // N_CORES  # 2048

# engine split knobs
MSPL = 39   # combine m-rows on DVE (of 64); rest on Pool
CSPL = 16   # mpre chunks on DVE (of 32); rest on Pool
ACT_COPIES = 3  # of 8 transpose-drain copies per tile go to Act
DVE_COPIES = 5  # rest on DVE (Pool/GPSIMD cannot access PSUM)

_BUILD_CACHE = {}


def _build(b_core, iters, with_bm1):
    """Build and compile the single-core Bass program."""
    import concourse.tile as tile
    import concourse.bacc as bacc
    import concourse.mybir as mybir
    from concourse import masks
    from contextlib import ExitStack

    f32 = mybir.dt.float32
    bf16 = mybir.dt.bfloat16
    Alu = mybir.AluOpType
    Act = mybir.ActivationFunctionType

    NT = b_core // 256  # tiles of 256 samples
    assert b_core % 256 == 0

    nc = bacc.Bacc("TRN2", target_bir_lowering=False, debug=False,
                   num_devices=N_CORES)

    # ---- DRAM tensors (host-prepped layouts) ----
    d_mem = nc.dram_tensor("mem", (NT, 128, 32 * 256), bf16, kind="ExternalInput")
    d_qa = nc.dram_tensor("qa", (NT, 128, 256), bf16, kind="ExternalInput")
    d_ck = nc.dram_tensor("ck", (NT, 64, 256), bf16, kind="ExternalInput")
    d_wc0 = nc.dram_tensor("wc0", (128, 32 * 128), bf16, kind="ExternalInput")
    d_wm1 = nc.dram_tensor("wm1", (128, 32 * 128), bf16, kind="ExternalInput")
    d_wez = nc.dram_tensor("wez", (128, 32 * 128), bf16, kind="ExternalInput")
    d_wamv = nc.dram_tensor("wamv", (128, 32 * 64), bf16, kind="ExternalInput")
    d_wewz = nc.dram_tensor("wewz", (128, 128), bf16, kind="ExternalInput")
    d_wza = nc.dram_tensor("wza", (DV, DV), bf16, kind="ExternalInput")
    d_mkt = nc.dram_tensor("mkt", (DK, M), bf16, kind="ExternalInput")
    d_bias = nc.dram_tensor("biasv", (128, 8), f32, kind="ExternalInput")
    if with_bm1:
        d_bm1 = nc.dram_tensor("bm1fb", (128, 32), f32, kind="ExternalInput")
    d_out = nc.dram_tensor("out", (b_core, F), bf16, kind="ExternalOutput")

    mem_r = d_mem.ap().rearrange("t p (c x) -> t p c x", c=32)
    out_r = d_out.ap().rearrange("(t s p) f -> t p s f", p=128, s=2)

    with tile.TileContext(nc) as tc:
        with ExitStack() as ctx:
            wpool = ctx.enter_context(tc.tile_pool(name="wpool", bufs=1))
            pmem = ctx.enter_context(tc.tile_pool(name="pmem", bufs=2))
            pqa = ctx.enter_context(tc.tile_pool(name="pqa", bufs=2))
            pgate = ctx.enter_context(tc.tile_pool(name="pgate", bufs=2))
            pmpre = ctx.enter_context(tc.tile_pool(name="pmpre", bufs=2))
            pnat = ctx.enter_context(tc.tile_pool(name="pnat", bufs=2))
            pt1 = ctx.enter_context(tc.tile_pool(name="pt1", bufs=1))
            psml = ctx.enter_context(tc.tile_pool(name="psml", bufs=2))
            ps_c0 = ctx.enter_context(tc.tile_pool(name="ps_c0", bufs=1, space="PSUM"))
            ps_g = ctx.enter_context(tc.tile_pool(name="ps_g", bufs=2, space="PSUM"))
            ps_mv = ctx.enter_context(tc.tile_pool(name="ps_mv", bufs=1, space="PSUM"))
            ps_sml = ctx.enter_context(tc.tile_pool(name="ps_sml", bufs=1, space="PSUM"))
            ps_tp = ctx.enter_context(tc.tile_pool(name="ps_tp", bufs=2, space="PSUM"))

            # ---- weights into SBUF (once) ----
            w_c0 = wpool.tile([128, 32, 128], bf16, tag="w_c0")
            nc.sync.dma_start(w_c0[:], d_wc0.ap().rearrange("k (c q) -> k c q", c=32))
            w_m1 = wpool.tile([128, 32, 128], bf16, tag="w_m1")
            nc.sync.dma_start(w_m1[:], d_wm1.ap().rearrange("k (c q) -> k c q", c=32))
            w_ez = wpool.tile([128, 32, 128], bf16, tag="w_ez")
            nc.sync.dma_start(w_ez[:], d_wez.ap().rearrange("k (c q) -> k c q", c=32))
            w_amv = wpool.tile([128, 32, 64], bf16, tag="w_amv")
            nc.sync.dma_start(w_amv[:], d_wamv.ap().rearrange("k (c q) -> k c q", c=32))
            w_ewz = wpool.tile([128, 128], bf16, tag="w_ewz")
            nc.sync.dma_start(w_ewz[:], d_wewz.ap())
            w_za = wpool.tile([DV, DV], bf16, tag="w_za")
            nc.sync.dma_start(w_za[:], d_wza.ap())
            w_mkt = wpool.tile([DK, M], bf16, tag="w_mkt")
            nc.sync.dma_start(w_mkt[:], d_mkt.ap())
            biasv = wpool.tile([128, 8], f32, tag="biasv")
            nc.sync.dma_start(biasv[:], d_bias.ap())
            ck_all = wpool.tile([64, NT, 256], bf16, tag="ck_all")
            nc.sync.dma_start(ck_all[:], d_ck.ap().rearrange("t p x -> p t x"))
            if with_bm1:
                bm1fb = wpool.tile([128, 32], f32, tag="bm1fb")
                nc.sync.dma_start(bm1fb[:], d_bm1.ap())
            ident = wpool.tile([128, 128], bf16, tag="ident")
            masks.make_identity(nc, ident[:])
            w2_all = wpool.tile([128, NT, 2, 128], bf16, tag="w2_all")

            bc0 = biasv[:, 0:1]
            b_e = biasv[0:64, 1:2]
            b_z = biasv[0:64, 2:3]
            b_emv = biasv[0:64, 3:4]
            b_zmv = biasv[0:64, 4:5]
            b_amv = biasv[0:64, 5:6]
            b_za = biasv[0:64, 6:7]

            def prologue():
                """softmax(ck @ mk^T) for all tiles -> w2_all, pair-duplicated."""
                for t in range(NT):
                    lgt = ps_sml.tile([128, 2, 256], f32, tag="smlps")
                    lg = lgt[:, 1, 0:128].rearrange("p (s m) -> p s m", s=2)
                    for s in range(2):
                        nc.tensor.matmul(lg[:, s], ck_all[:, t, s * 128:(s + 1) * 128],
                                         w_mkt[:], start=True, stop=True)
                    # drain psum fast so the shared bank cycles quickly
                    lgs = psml.tile([128, 2, 64], f32, tag="lgs")
                    nc.vector.tensor_copy(lgs[:], lg)
                    # |logit| <~ 40 so f32 exp without max-subtraction is safe
                    exv = psml.tile([128, 2, 64], f32, tag="exv")
                    nc.scalar.activation(exv[:], lgs[:], Act.Exp)
                    sm = psml.tile([128, 2, 1], f32, tag="sm")
                    nc.vector.tensor_reduce(sm[:], exv[:],
                                            mybir.AxisListType.X, Alu.add)
                    nc.vector.reciprocal(sm[:], sm[:])
                    for s in range(2):
                        dst = w2_all[:, t, s, :].rearrange("p (m r) -> p m r", r=2)
                        nc.vector.tensor_scalar_mul(
                            dst, exv[:, s].unsqueeze(2).broadcast_to([128, 64, 2]),
                            sm[:, s])

            def load_tile(t):
                memf = pmem.tile([128, 32, 256], bf16, tag="memf")
                nc.sync.dma_start(memf[:], mem_r[t])
                qaT = pqa.tile([128, 256], bf16, tag="qaT")
                nc.sync.dma_start(qaT[:], d_qa.ap()[t])
                return memf, qaT

            def phase_a(t, loaded):
                """c0 + gate, all in feature-major [f, b] layout."""
                memf, qaT = loaded
                c0ps = ps_c0.tile([128, 256], f32, tag="c0")
                for c in range(32):
                    nc.tensor.matmul(c0ps[:], w_c0[:, c, :], memf[:, c, :],
                                     start=(c == 0), stop=(c == 31))
                c0s = psml.tile([128, 256], bf16, tag="c0s")
                nc.scalar.activation(c0s[:], c0ps[:], Act.Sigmoid, bias=bc0)
                c0q = psml.tile([128, 256], bf16, tag="c0q")
                nc.vector.tensor_tensor(c0q[:], c0s[:], qaT[:], op=Alu.mult)

                gate = pgate.tile([128, 32, 256], bf16, tag="gate")
                for g in range(16):
                    gps = ps_g.tile([128, 2, 256], f32, tag="g")
                    for cc in range(2):
                        c = 2 * g + cc
                        nc.tensor.matmul(gps[:, cc], w_m1[:, c, :], c0q[:],
                                         start=True, stop=True)
                    if with_bm1:
                        for cc in range(2):
                            c = 2 * g + cc
                            nc.scalar.activation(gate[:, c, :], gps[:, cc],
                                                 Act.Sigmoid, bias=bm1fb[:, c:c + 1])
                    else:
                        nc.scalar.activation(gate[:, 2 * g:2 * g + 2, :], gps[:],
                                             Act.Sigmoid)
                return dict(memf=memf, c0q=c0q, gate=gate)

            def phase_m(st):
                """mpre = mem * gate in [f, b], split DVE/Pool."""
                memf, gate = st["memf"], st["gate"]
                mpre = pmpre.tile([128, 32, 256], bf16, tag="mpre")
                nc.vector.tensor_tensor(mpre[:, 0:CSPL], memf[:, 0:CSPL],
                                        gate[:, 0:CSPL], op=Alu.mult)
                nc.gpsimd.tensor_tensor(mpre[:, CSPL:32], memf[:, CSPL:32],
                                        gate[:, CSPL:32], op=Alu.mult)
                st["mpre"] = mpre
                return st

            def phase_b(t, st):
                mpre, c0q = st["mpre"], st["c0q"]

                # ---- mv GEMMs from mpre_fb ----
                # av is 64-out: run chunk pairs concurrently in the two
                # column halves of the PE array (tile_position col tiling),
                # then sum the two psum halves.  ez and av live in separate
                # psum tiles: the scheduler may interleave their chains and
                # psum group tracking is per-memref.
                ezf = ps_mv.tile([128, 2, 256], f32, tag="ez")  # full-bank pad
                ezt = ezf[:, 0, :]
                ez = ezt
                avf = ps_mv.tile([128, 2, 256], f32, tag="av")  # full-bank pad
                av = avf[0:64, 0, :]
                for c in range(32):
                    nc.tensor.matmul(ez, w_ez[:, c, :], mpre[:, c, :],
                                     start=(c == 0), stop=(c == 31))
                for c in range(32):
                    nc.tensor.matmul(av, w_amv[:, c, :], mpre[:, c, :],
                                     start=(c == 0), stop=(c == 31))
                emvT = psml.tile([64, 256], bf16, tag="emvT")
                nc.scalar.activation(emvT[:], ezt[0:64], Act.Sigmoid,
                                     bias=b_emv)
                amvT = psml.tile([64, 256], bf16, tag="amvT")
                nc.scalar.activation(amvT[:], av, Act.Tanh, bias=b_amv)
                zmv = psml.tile([64, 256], bf16, tag="zmv")
                nc.scalar.activation(zmv[:], ezt[64:128], Act.Identity,
                                     bias=b_zmv)

                # ---- small epilogue chain ([feat, b]) ----
                smlps = ps_sml.tile([128, 2, 256], f32, tag="smlps")
                wz = smlps[:, 0, :]
                za = smlps[0:64, 1, :]
                nc.tensor.matmul(wz, w_ewz[:], c0q[:], start=True, stop=True)
                ecT = psml.tile([64, 256], bf16, tag="ecT")
                nc.scalar.activation(ecT[:], smlps[0:64, 0, :], Act.Sigmoid,
                                     bias=b_e)
                esum = psml.tile([64, 256], bf16, tag="esum")
                nc.vector.tensor_tensor(esum[:], ecT[:], emvT[:], op=Alu.add)
                eT = psml.tile([64, 256], bf16, tag="eT")
                nc.scalar.activation(eT[:], esum[:], Act.Sigmoid)
                zsum = psml.tile([64, 256], bf16, tag="zc")
                nc.vector.scalar_tensor_tensor(zsum[:], smlps[64:128, 0, :], b_z,
                                               zmv[:], op0=Alu.add, op1=Alu.add)
                ztT = psml.tile([64, 256], bf16, tag="ecT")
                nc.scalar.activation(ztT[:], zsum[:], Act.Sigmoid)
                nc.tensor.matmul(za, w_za[:], ztT[:], start=True, stop=True)
                zaT = psml.tile([64, 256], bf16, tag="zmv")
                nc.scalar.activation(zaT[:], za, Act.Tanh, bias=b_za)
                asum = psml.tile([64, 256], bf16, tag="asum")
                nc.vector.tensor_tensor(asum[:], zaT[:], amvT[:], op=Alu.add)
                aT = psml.tile([64, 256], bf16, tag="asum")
                nc.scalar.activation(aT[:], asum[:], Act.Tanh)

                # e/a -> batch-major [128, s, (e|a)]
                ea_nat = psml.tile([128, 2, 128], bf16, tag="ea_nat")
                te_t = ps_tp.tile([128, 1024], bf16, tag="tp")
                te = te_t[:, 0:256]
                for s in range(2):
                    base = s * 128
                    nc.tensor.transpose(te[:, base:base + 64],
                                        eT[:, s * 128:(s + 1) * 128],
                                        ident[0:64, 0:64])
                    nc.tensor.transpose(te[:, base + 64:base + 128],
                                        aT[:, s * 128:(s + 1) * 128],
                                        ident[0:64, 0:64])
                    nc.vector.tensor_copy(ea_nat[:, s],
                                          te[:, base:base + 128])

                # ---- mpre transposes + PSUM drains ----
                mpn = pnat.tile([128, 2, 4096], bf16, tag="mpn")
                ncopy = 0
                for s in range(2):
                    for g2 in range(4):
                        tp = ps_tp.tile([128, 1024], bf16, tag="tp")
                        for c8 in range(8):
                            c = g2 * 8 + c8
                            nc.tensor.transpose(
                                tp[:, c8 * 128:(c8 + 1) * 128],
                                mpre[:, c, s * 128:(s + 1) * 128], ident[:])
                        dst = mpn[:, s, g2 * 1024:(g2 + 1) * 1024]
                        if ncopy < ACT_COPIES:
                            nc.scalar.copy(dst, tp[:])
                        else:
                            nc.vector.tensor_copy(dst, tp[:])
                        ncopy += 1

                # ---- combine: new = mpre + w*(a - mpre*e) (batch-major) ----
                t1 = pt1.tile([128, 2, 4096], bf16, tag="t1")
                for s in range(2):
                    mp = mpn[:, s].rearrange("p (m d) -> p m d", m=64)
                    tv = t1[:, s].rearrange("p (m d) -> p m d", m=64)
                    tv4 = t1[:, s].rearrange("p (m g r) -> p m g r", m=64, r=2)
                    ebig = (ea_nat[:, s, 0:64].unsqueeze(1)
                            .broadcast_to([128, 64, 64]))
                    abig = (ea_nat[:, s, 64:128].unsqueeze(1)
                            .broadcast_to([128, 64, 64]))
                    w4 = (w2_all[:, t, s, :]
                          .rearrange("p (m r) -> p m r", r=2)
                          .unsqueeze(2).broadcast_to([128, 64, 32, 2]))
                    for eng, lo, hi in ((nc.vector, 0, MSPL),
                                        (nc.gpsimd, MSPL, 64)):
                        eng.tensor_tensor(tv[:, lo:hi], mp[:, lo:hi],
                                          ebig[:, lo:hi], op=Alu.mult)
                        eng.tensor_tensor(tv[:, lo:hi], abig[:, lo:hi],
                                          tv[:, lo:hi], op=Alu.subtract)
                        eng.tensor_tensor(tv4[:, lo:hi], tv4[:, lo:hi],
                                          w4[:, lo:hi], op=Alu.mult)
                        eng.tensor_tensor(mp[:, lo:hi], mp[:, lo:hi],
                                          tv[:, lo:hi], op=Alu.add)

                # ---- store (SP HWDGE; loads are issued earlier on the FIFO) ----
                nc.sync.dma_start(out_r[t], mpn[:])

            def whole():
                prologue()
                st = phase_a(0, load_tile(0))
                st = phase_m(st)
                for t in range(NT):
                    st_next = None
                    if t + 1 < NT:
                        st_next = phase_a(t + 1, load_tile(t + 1))
                    phase_b(t, st)
                    if st_next is not None:
                        st = phase_m(st_next)

            if iters == 1:
                whole()
            else:
                with tc.For_i(0, iters, 1,
                              hint_engines=(mybir.EngineType.PE,
                                            mybir.EngineType.DVE,
                                            mybir.EngineType.Activation,
                                            mybir.EngineType.Pool,
                                            mybir.EngineType.SP)):
                    whole()

    nc.compile()
    return nc


def _get_nc(b_core, iters, with_bm1):
    key = (b_core, iters, with_bm1)
    if key not in _BUILD_CACHE:
        _BUILD_CACHE[key] = _build(b_core, iters, with_bm1)
    return _BUILD_CACHE[key]


def _prep_weights(inputs):
    bf = ml_dtypes.bfloat16
    wc0 = np.ascontiguousarray(
        inputs["Wc0"].reshape(32, 128, 128).transpose(1, 0, 2).reshape(128, -1)
    ).astype(bf)
    wez_full = np.concatenate([inputs["Wemv"], inputs["Wzmv"]], axis=1)
    wez = np.ascontiguousarray(
        wez_full.reshape(32, 128, 128).transpose(1, 0, 2).reshape(128, -1)
    ).astype(bf)
    wamv = np.ascontiguousarray(
        inputs["Wamv"].reshape(32, 128, 64).transpose(1, 0, 2).reshape(128, -1)
    ).astype(bf)
    wewz = np.concatenate([inputs["We"], inputs["Wz"]], axis=1).astype(bf)
    wm1 = np.ascontiguousarray(inputs["Wm1"]).astype(bf)  # [128, 4096] chunk-major
    wza = inputs["Wza"].astype(bf)
    mkt = np.ascontiguousarray(inputs["memory_key"].T).astype(bf)

    biasv = np.zeros((128, 8), np.float32)
    biasv[:, 0] = inputs["bc0"]
    biasv[0:64, 1] = inputs["be"]
    biasv[0:64, 2] = inputs["bz"]
    biasv[0:64, 3] = inputs["bemv"]
    biasv[0:64, 4] = inputs["bzmv"]
    biasv[0:64, 5] = inputs["bamv"]
    biasv[0:64, 6] = inputs["bza"]

    w = dict(wc0=wc0, wm1=wm1, wez=wez, wamv=wamv, wewz=wewz, wza=wza,
             mkt=mkt, biasv=biasv)
    with_bm1 = bool(np.any(inputs["bm1"]))
    if with_bm1:
        w["bm1fb"] = np.ascontiguousarray(
            inputs["bm1"].reshape(32, 128).T).astype(np.float32)
    return w, with_bm1


def _make_in_maps(inputs, b_core):
    bf = ml_dtypes.bfloat16
    NT = b_core // 256
    wdict, _ = _prep_weights(inputs)
    mem = np.asarray(inputs["memory_value"]).reshape(B, F)
    qa = np.asarray(inputs["control_qa"])
    ck = np.asarray(inputs["control_key"])
    in_maps = []
    for c in range(N_CORES):
        sl = slice(c * b_core, (c + 1) * b_core)
        memc = mem[sl].astype(bf)
        mem_t = np.ascontiguousarray(
            memc.reshape(NT, 256, 32, 128).transpose(0, 3, 2, 1)
        ).reshape(NT, 128, 32 * 256)
        qa_t = np.ascontiguousarray(
            qa[sl].astype(bf).reshape(NT, 256, DQA).transpose(0, 2, 1))
        ck_t = np.ascontiguousarray(
            ck[sl].astype(bf).reshape(NT, 256, DK).transpose(0, 2, 1))
        in_maps.append(dict(mem=mem_t, qa=qa_t, ck=ck_t, **wdict))
    return in_maps


def kernel(**inputs):
    from concourse import bass_utils
    inputs = {k: np.asarray(v) for k, v in inputs.items()}
    _, with_bm1 = _prep_weights(inputs)
    nc = _get_nc(B_CORE, 1, with_bm1)
    in_maps = _make_in_maps(inputs, B_CORE)
    res = bass_utils.run_bass_kernel_spmd(nc, in_maps, core_ids=list(range(N_CORES)))
    out = np.concatenate([r["out"] for r in res.results], axis=0)
    return out.astype(np.float32).reshape(B, M, DV)


# revision 37
# speedup vs baseline: 1.5689x; 1.5689x over previous
# Trainium2 Bass kernel for nn_MEMORY_34986803593776 (scatter_memory).
#
# Math (per sample b):
#   w        = softmax(ck @ mk^T)                             [M]
#   c0       = qa * sigmoid(mem0 @ Wc0 + bc0)                 [DQA]
#   gate     = sigmoid(c0 @ Wm1 + bm1)                        [M*DV]
#   memPre   = mem0 * gate                                    [M*DV]
#   erase    = sig(sig(c0@We+be) + sig(memPre@Wemv+bemv))     [DV]
#   zt       = sig((c0@Wz+bz) + (memPre@Wzmv+bzmv))           [DV]
#   add      = tanh(tanh(zt@Wza+bza) + tanh(memPre@Wamv+bamv))[DV]
#   new      = memPre*(1 - w[m]*erase[dv]) + w[m]*add[dv]     [M,DV]
#
# Sharding: pure data parallel over batch B=16384 across 8 cores (2048/core).
#
# v2 layout strategy:
#  - Host pre-transposes mem to feature-major [f, b] bf16 tiles, so every
#    f-contraction GEMM (c0, gate, emv/zmv, amv) consumes it directly with
#    zero on-device input transposes.  qa/ck also arrive pre-transposed.
#  - Only memPre is PE-transposed back to batch-major for the combine;
#    PSUM->SBUF drains are spread across Act/DVE.
#  - The big elementwise passes (memPre mult + 4 combine passes) are
#    column-split between DVE and Pool(gpsimd).
#  - All DMAs are same-dtype bf16 on the SP queue (HWDGE), keeping the
#    Pool queue free for compute; output is bf16, upcast on host.

import numpy as np
import ml_dtypes

B = 16384
M = 64
DV = 64
DK = 64
DQA = 128
F = M * DV  # 4096
N_CORES = 8
B_CORE = B // N_CORES  # 2048

# engine split knobs
# Real-HW calibration: GPSIMD/Pool tensor ops run ~4-6x slower than the
# CoreSim model (software Q7 loop), so Pool gets no bulk elementwise work.
MSPL = 64   # combine m-rows on DVE (of 64); rest on Pool
CSPL = 32   # mpre chunks on DVE (of 32); rest on Pool
ACT_COPIES = 8  # of 8 transpose-drain copies per tile go to Act
DVE_COPIES = 0  # rest on DVE (Pool/GPSIMD cannot access PSUM)

_BUILD_CACHE = {}


def _build(b_core, iters, with_bm1):
    """Build and compile the single-core Bass program."""
    import concourse.tile as tile
    import concourse.bacc as bacc
    import concourse.mybir as mybir
    from concourse import masks
    from contextlib import ExitStack

    f32 = mybir.dt.float32
    bf16 = mybir.dt.bfloat16
    Alu = mybir.AluOpType
    Act = mybir.ActivationFunctionType

    NT = b_core // 256  # tiles of 256 samples
    assert b_core % 256 == 0

    nc = bacc.Bacc("TRN2", target_bir_lowering=False, debug=False,
                   num_devices=N_CORES)

    # ---- DRAM tensors (host-prepped layouts) ----
    d_mem = nc.dram_tensor("mem", (NT, 128, 32 * 256), bf16, kind="ExternalInput")
    d_qa = nc.dram_tensor("qa", (NT, 128, 256), bf16, kind="ExternalInput")
    d_ck = nc.dram_tensor("ck", (NT, 64, 256), bf16, kind="ExternalInput")
    d_wc0 = nc.dram_tensor("wc0", (128, 32 * 128), bf16, kind="ExternalInput")
    d_wm1 = nc.dram_tensor("wm1", (128, 32 * 128), bf16, kind="ExternalInput")
    d_wez = nc.dram_tensor("wez", (128, 32 * 128), bf16, kind="ExternalInput")
    d_wamv = nc.dram_tensor("wamv", (128, 32 * 64), bf16, kind="ExternalInput")
    d_wewz = nc.dram_tensor("wewz", (128, 128), bf16, kind="ExternalInput")
    d_wza = nc.dram_tensor("wza", (DV, DV), bf16, kind="ExternalInput")
    d_mkt = nc.dram_tensor("mkt", (DK, M), bf16, kind="ExternalInput")
    d_bias = nc.dram_tensor("biasv", (128, 8), f32, kind="ExternalInput")
    if with_bm1:
        d_bm1 = nc.dram_tensor("bm1fb", (128, 32), f32, kind="ExternalInput")
    d_out = nc.dram_tensor("out", (b_core, F), bf16, kind="ExternalOutput")

    mem_r = d_mem.ap().rearrange("t p (c x) -> t p c x", c=32)
    out_r = d_out.ap().rearrange("(t s p) f -> t p s f", p=128, s=2)

    with tile.TileContext(nc) as tc:
        with ExitStack() as ctx:
            wpool = ctx.enter_context(tc.tile_pool(name="wpool", bufs=1))
            pmem = ctx.enter_context(tc.tile_pool(name="pmem", bufs=2))
            pqa = ctx.enter_context(tc.tile_pool(name="pqa", bufs=2))
            pgate = ctx.enter_context(tc.tile_pool(name="pgate", bufs=2))
            pmpre = ctx.enter_context(tc.tile_pool(name="pmpre", bufs=2))
            pnat = ctx.enter_context(tc.tile_pool(name="pnat", bufs=2))
            pt1 = ctx.enter_context(tc.tile_pool(name="pt1", bufs=1))
            psml = ctx.enter_context(tc.tile_pool(name="psml", bufs=2))
            ps_c0 = ctx.enter_context(tc.tile_pool(name="ps_c0", bufs=1, space="PSUM"))
            ps_g = ctx.enter_context(tc.tile_pool(name="ps_g", bufs=2, space="PSUM"))
            ps_mv = ctx.enter_context(tc.tile_pool(name="ps_mv", bufs=1, space="PSUM"))
            ps_sml = ctx.enter_context(tc.tile_pool(name="ps_sml", bufs=1, space="PSUM"))
            ps_tp = ctx.enter_context(tc.tile_pool(name="ps_tp", bufs=2, space="PSUM"))

            # ---- weights into SBUF (once) ----
            w_c0 = wpool.tile([128, 32, 128], bf16, tag="w_c0")
            nc.sync.dma_start(w_c0[:], d_wc0.ap().rearrange("k (c q) -> k c q", c=32))
            w_m1 = wpool.tile([128, 32, 128], bf16, tag="w_m1")
            nc.sync.dma_start(w_m1[:], d_wm1.ap().rearrange("k (c q) -> k c q", c=32))
            w_ez = wpool.tile([128, 32, 128], bf16, tag="w_ez")
            nc.sync.dma_start(w_ez[:], d_wez.ap().rearrange("k (c q) -> k c q", c=32))
            w_amv = wpool.tile([128, 32, 64], bf16, tag="w_amv")
            nc.sync.dma_start(w_amv[:], d_wamv.ap().rearrange("k (c q) -> k c q", c=32))
            w_ewz = wpool.tile([128, 128], bf16, tag="w_ewz")
            nc.sync.dma_start(w_ewz[:], d_wewz.ap())
            w_za = wpool.tile([DV, DV], bf16, tag="w_za")
            nc.sync.dma_start(w_za[:], d_wza.ap())
            w_mkt = wpool.tile([DK, M], bf16, tag="w_mkt")
            nc.sync.dma_start(w_mkt[:], d_mkt.ap())
            biasv = wpool.tile([128, 8], f32, tag="biasv")
            nc.sync.dma_start(biasv[:], d_bias.ap())
            ck_all = wpool.tile([64, NT, 256], bf16, tag="ck_all")
            nc.sync.dma_start(ck_all[:], d_ck.ap().rearrange("t p x -> p t x"))
            if with_bm1:
                bm1fb = wpool.tile([128, 32], f32, tag="bm1fb")
                nc.sync.dma_start(bm1fb[:], d_bm1.ap())
            ident = wpool.tile([128, 128], bf16, tag="ident")
            masks.make_identity(nc, ident[:])
            w2_all = wpool.tile([128, NT, 2, 128], bf16, tag="w2_all")

            bc0 = biasv[:, 0:1]
            b_e = biasv[0:64, 1:2]
            b_z = biasv[0:64, 2:3]
            b_emv = biasv[0:64, 3:4]
            b_zmv = biasv[0:64, 4:5]
            b_amv = biasv[0:64, 5:6]
            b_za = biasv[0:64, 6:7]

            def prologue():
                """softmax(ck @ mk^T) for all tiles -> w2_all, pair-duplicated."""
                for t in range(NT):
                    lgt = ps_sml.tile([128, 2, 256], f32, tag="smlps")
                    lg = lgt[:, 1, 0:128].rearrange("p (s m) -> p s m", s=2)
                    for s in range(2):
                        nc.tensor.matmul(lg[:, s], ck_all[:, t, s * 128:(s + 1) * 128],
                                         w_mkt[:], start=True, stop=True)
                    # drain psum fast so the shared bank cycles quickly
                    lgs = psml.tile([128, 2, 64], f32, tag="lgs")
                    nc.vector.tensor_copy(lgs[:], lg)
                    # |logit| <~ 40 so f32 exp without max-subtraction is safe
                    exv = psml.tile([128, 2, 64], f32, tag="exv")
                    nc.scalar.activation(exv[:], lgs[:], Act.Exp)
                    sm = psml.tile([128, 2, 1], f32, tag="sm")
                    nc.vector.tensor_reduce(sm[:], exv[:],
                                            mybir.AxisListType.X, Alu.add)
                    nc.vector.reciprocal(sm[:], sm[:])
                    for s in range(2):
                        dst = w2_all[:, t, s, :].rearrange("p (m r) -> p m r", r=2)
                        nc.vector.tensor_scalar_mul(
                            dst, exv[:, s].unsqueeze(2).broadcast_to([128, 64, 2]),
                            sm[:, s])

            def load_tile(t):
                memf = pmem.tile([128, 32, 256], bf16, tag="memf")
                nc.sync.dma_start(memf[:], mem_r[t])
                qaT = pqa.tile([128, 256], bf16, tag="qaT")
                nc.sync.dma_start(qaT[:], d_qa.ap()[t])
                return memf, qaT

            def phase_a(t, loaded):
                """c0 + gate, all in feature-major [f, b] layout."""
                memf, qaT = loaded
                c0ps = ps_c0.tile([128, 256], f32, tag="c0")
                for c in range(32):
                    nc.tensor.matmul(c0ps[:], w_c0[:, c, :], memf[:, c, :],
                                     start=(c == 0), stop=(c == 31))
                c0s = psml.tile([128, 256], bf16, tag="c0s")
                nc.scalar.activation(c0s[:], c0ps[:], Act.Sigmoid, bias=bc0)
                c0q = psml.tile([128, 256], bf16, tag="c0q")
                nc.gpsimd.tensor_tensor(c0q[:], c0s[:], qaT[:], op=Alu.mult)

                gate = pgate.tile([128, 32, 256], bf16, tag="gate")
                for g in range(16):
                    gps = ps_g.tile([128, 2, 256], f32, tag="g")
                    for cc in range(2):
                        c = 2 * g + cc
                        nc.tensor.matmul(gps[:, cc], w_m1[:, c, :], c0q[:],
                                         start=True, stop=True)
                    if with_bm1:
                        for cc in range(2):
                            c = 2 * g + cc
                            nc.scalar.activation(gate[:, c, :], gps[:, cc],
                                                 Act.Sigmoid, bias=bm1fb[:, c:c + 1])
                    else:
                        nc.scalar.activation(gate[:, 2 * g:2 * g + 2, :], gps[:],
                                             Act.Sigmoid)
                return dict(memf=memf, c0q=c0q, gate=gate)

            def phase_m(st):
                """mpre = mem * gate in [f, b], split DVE/Pool."""
                memf, gate = st["memf"], st["gate"]
                mpre = pmpre.tile([128, 32, 256], bf16, tag="mpre")
                nc.vector.tensor_tensor(mpre[:, 0:CSPL], memf[:, 0:CSPL],
                                        gate[:, 0:CSPL], op=Alu.mult)
                if CSPL < 32:
                    nc.gpsimd.tensor_tensor(mpre[:, CSPL:32], memf[:, CSPL:32],
                                            gate[:, CSPL:32], op=Alu.mult)
                st["mpre"] = mpre
                return st

            def phase_b(t, st):
                mpre, c0q = st["mpre"], st["c0q"]

                # ---- mv GEMMs from mpre_fb ----
                # av is 64-out: run chunk pairs concurrently in the two
                # column halves of the PE array (tile_position col tiling),
                # then sum the two psum halves.  ez and av live in separate
                # psum tiles: the scheduler may interleave their chains and
                # psum group tracking is per-memref.
                ezf = ps_mv.tile([128, 2, 256], f32, tag="ez")  # full-bank pad
                ezt = ezf[:, 0, :]
                ez = ezt
                avf = ps_mv.tile([128, 2, 256], f32, tag="av")  # full-bank pad
                av = avf[0:64, 0, :]
                for c in range(32):
                    nc.tensor.matmul(ez, w_ez[:, c, :], mpre[:, c, :],
                                     start=(c == 0), stop=(c == 31))
                for c in range(32):
                    nc.tensor.matmul(av, w_amv[:, c, :], mpre[:, c, :],
                                     start=(c == 0), stop=(c == 31))
                emvT = psml.tile([64, 256], bf16, tag="emvT")
                nc.scalar.activation(emvT[:], ezt[0:64], Act.Sigmoid,
                                     bias=b_emv)
                amvT = psml.tile([64, 256], bf16, tag="amvT")
                nc.scalar.activation(amvT[:], av, Act.Tanh, bias=b_amv)
                zmv = psml.tile([64, 256], bf16, tag="zmv")
                nc.scalar.activation(zmv[:], ezt[64:128], Act.Identity,
                                     bias=b_zmv)

                # ---- small epilogue chain ([feat, b]) ----
                smlps = ps_sml.tile([128, 2, 256], f32, tag="smlps")
                wz = smlps[:, 0, :]
                za = smlps[0:64, 1, :]
                nc.tensor.matmul(wz, w_ewz[:], c0q[:], start=True, stop=True)
                ecT = psml.tile([64, 256], bf16, tag="ecT")
                nc.scalar.activation(ecT[:], smlps[0:64, 0, :], Act.Sigmoid,
                                     bias=b_e)
                esum = psml.tile([64, 256], bf16, tag="esum")
                nc.gpsimd.tensor_tensor(esum[:], ecT[:], emvT[:], op=Alu.add)
                eT = psml.tile([64, 256], bf16, tag="eT")
                nc.scalar.activation(eT[:], esum[:], Act.Sigmoid)
                zsum = psml.tile([64, 256], bf16, tag="zc")
                nc.vector.scalar_tensor_tensor(zsum[:], smlps[64:128, 0, :], b_z,
                                               zmv[:], op0=Alu.add, op1=Alu.add)
                ztT = psml.tile([64, 256], bf16, tag="ecT")
                nc.scalar.activation(ztT[:], zsum[:], Act.Sigmoid)
                nc.tensor.matmul(za, w_za[:], ztT[:], start=True, stop=True)
                zaT = psml.tile([64, 256], bf16, tag="zmv")
                nc.scalar.activation(zaT[:], za, Act.Tanh, bias=b_za)
                asum = psml.tile([64, 256], bf16, tag="asum")
                nc.gpsimd.tensor_tensor(asum[:], zaT[:], amvT[:], op=Alu.add)
                aT = psml.tile([64, 256], bf16, tag="asum")
                nc.scalar.activation(aT[:], asum[:], Act.Tanh)

                # e/a -> batch-major [128, s, (e|a)]
                ea_nat = psml.tile([128, 2, 128], bf16, tag="ea_nat")
                te_t = ps_tp.tile([128, 1024], bf16, tag="tp")
                te = te_t[:, 0:256]
                for s in range(2):
                    base = s * 128
                    nc.tensor.transpose(te[:, base:base + 64],
                                        eT[:, s * 128:(s + 1) * 128],
                                        ident[0:64, 0:64])
                    nc.tensor.transpose(te[:, base + 64:base + 128],
                                        aT[:, s * 128:(s + 1) * 128],
                                        ident[0:64, 0:64])
                    nc.vector.tensor_copy(ea_nat[:, s],
                                          te[:, base:base + 128])

                # ---- mpre transposes + PSUM drains ----
                mpn = pnat.tile([128, 2, 4096], bf16, tag="mpn")
                ncopy = 0
                for s in range(2):
                    for g2 in range(4):
                        tp = ps_tp.tile([128, 1024], bf16, tag="tp")
                        for c8 in range(8):
                            c = g2 * 8 + c8
                            nc.tensor.transpose(
                                tp[:, c8 * 128:(c8 + 1) * 128],
                                mpre[:, c, s * 128:(s + 1) * 128], ident[:])
                        dst = mpn[:, s, g2 * 1024:(g2 + 1) * 1024]
                        if ncopy < ACT_COPIES:
                            nc.scalar.copy(dst, tp[:])
                        else:
                            nc.vector.tensor_copy(dst, tp[:])
                        ncopy += 1

                # ---- combine: new = mpre + w*(a - mpre*e) (batch-major) ----
                t1 = pt1.tile([128, 2, 4096], bf16, tag="t1")
                for s in range(2):
                    mp = mpn[:, s].rearrange("p (m d) -> p m d", m=64)
                    tv = t1[:, s].rearrange("p (m d) -> p m d", m=64)
                    tv4 = t1[:, s].rearrange("p (m g r) -> p m g r", m=64, r=2)
                    ebig = (ea_nat[:, s, 0:64].unsqueeze(1)
                            .broadcast_to([128, 64, 64]))
                    abig = (ea_nat[:, s, 64:128].unsqueeze(1)
                            .broadcast_to([128, 64, 64]))
                    w4 = (w2_all[:, t, s, :]
                          .rearrange("p (m r) -> p m r", r=2)
                          .unsqueeze(2).broadcast_to([128, 64, 32, 2]))
                    for eng, lo, hi in ((nc.vector, 0, MSPL),
                                        (nc.gpsimd, MSPL, 64)):
                        if lo == hi:
                            continue
                        eng.tensor_tensor(tv[:, lo:hi], mp[:, lo:hi],
                                          ebig[:, lo:hi], op=Alu.mult)
                        eng.tensor_tensor(tv[:, lo:hi], abig[:, lo:hi],
                                          tv[:, lo:hi], op=Alu.subtract)
                        eng.tensor_tensor(tv4[:, lo:hi], tv4[:, lo:hi],
                                          w4[:, lo:hi], op=Alu.mult)
                        eng.tensor_tensor(mp[:, lo:hi], mp[:, lo:hi],
                                          tv[:, lo:hi], op=Alu.add)

                # ---- store (SP HWDGE; loads are issued earlier on the FIFO) ----
                nc.sync.dma_start(out_r[t], mpn[:])

            def whole():
                prologue()
                st = phase_a(0, load_tile(0))
                st = phase_m(st)
                for t in range(NT):
                    st_next = None
                    if t + 1 < NT:
                        st_next = phase_a(t + 1, load_tile(t + 1))
                    phase_b(t, st)
                    if st_next is not None:
                        st = phase_m(st_next)

            if iters == 1:
                whole()
            else:
                with tc.For_i(0, iters, 1,
                              hint_engines=(mybir.EngineType.PE,
                                            mybir.EngineType.DVE,
                                            mybir.EngineType.Activation,
                                            mybir.EngineType.Pool,
                                            mybir.EngineType.SP)):
                    whole()

    nc.compile()
    return nc


def _get_nc(b_core, iters, with_bm1):
    key = (b_core, iters, with_bm1)
    if key not in _BUILD_CACHE:
        _BUILD_CACHE[key] = _build(b_core, iters, with_bm1)
    return _BUILD_CACHE[key]


def _prep_weights(inputs):
    bf = ml_dtypes.bfloat16
    wc0 = np.ascontiguousarray(
        inputs["Wc0"].reshape(32, 128, 128).transpose(1, 0, 2).reshape(128, -1)
    ).astype(bf)
    wez_full = np.concatenate([inputs["Wemv"], inputs["Wzmv"]], axis=1)
    wez = np.ascontiguousarray(
        wez_full.reshape(32, 128, 128).transpose(1, 0, 2).reshape(128, -1)
    ).astype(bf)
    wamv = np.ascontiguousarray(
        inputs["Wamv"].reshape(32, 128, 64).transpose(1, 0, 2).reshape(128, -1)
    ).astype(bf)
    wewz = np.concatenate([inputs["We"], inputs["Wz"]], axis=1).astype(bf)
    wm1 = np.ascontiguousarray(inputs["Wm1"]).astype(bf)  # [128, 4096] chunk-major
    wza = inputs["Wza"].astype(bf)
    mkt = np.ascontiguousarray(inputs["memory_key"].T).astype(bf)

    biasv = np.zeros((128, 8), np.float32)
    biasv[:, 0] = inputs["bc0"]
    biasv[0:64, 1] = inputs["be"]
    biasv[0:64, 2] = inputs["bz"]
    biasv[0:64, 3] = inputs["bemv"]
    biasv[0:64, 4] = inputs["bzmv"]
    biasv[0:64, 5] = inputs["bamv"]
    biasv[0:64, 6] = inputs["bza"]

    w = dict(wc0=wc0, wm1=wm1, wez=wez, wamv=wamv, wewz=wewz, wza=wza,
             mkt=mkt, biasv=biasv)
    with_bm1 = bool(np.any(inputs["bm1"]))
    if with_bm1:
        w["bm1fb"] = np.ascontiguousarray(
            inputs["bm1"].reshape(32, 128).T).astype(np.float32)
    return w, with_bm1


def _make_in_maps(inputs, b_core):
    bf = ml_dtypes.bfloat16
    NT = b_core // 256
    wdict, _ = _prep_weights(inputs)
    mem = np.asarray(inputs["memory_value"]).reshape(B, F)
    qa = np.asarray(inputs["control_qa"])
    ck = np.asarray(inputs["control_key"])
    in_maps = []
    for c in range(N_CORES):
        sl = slice(c * b_core, (c + 1) * b_core)
        memc = mem[sl].astype(bf)
        mem_t = np.ascontiguousarray(
            memc.reshape(NT, 256, 32, 128).transpose(0, 3, 2, 1)
        ).reshape(NT, 128, 32 * 256)
        qa_t = np.ascontiguousarray(
            qa[sl].astype(bf).reshape(NT, 256, DQA).transpose(0, 2, 1))
        ck_t = np.ascontiguousarray(
            ck[sl].astype(bf).reshape(NT, 256, DK).transpose(0, 2, 1))
        in_maps.append(dict(mem=mem_t, qa=qa_t, ck=ck_t, **wdict))
    return in_maps


def kernel(**inputs):
    from concourse import bass_utils
    inputs = {k: np.asarray(v) for k, v in inputs.items()}
    _, with_bm1 = _prep_weights(inputs)
    nc = _get_nc(B_CORE, 1, with_bm1)
    in_maps = _make_in_maps(inputs, B_CORE)
    res = bass_utils.run_bass_kernel_spmd(nc, in_maps, core_ids=list(range(N_CORES)))
    out = np.concatenate([r["out"] for r in res.results], axis=0)
    return out.astype(np.float32).reshape(B, M, DV)
